# revision 30
# baseline (speedup 1.0000x reference)
"""Trainium2 Bass kernel for fused AdaRMSNorm + QK-RMSNorm/RoPE attention.

Sharding: 8 cores = 2 batch groups x 4 head-groups (8 heads each).

Host prep (per call, numpy):
  rr = rsqrt(mean(x^2)+eps) per (b,l); xs = x*rr  (adaRMS scale folded into W,
  shift folded into per-j bias c).  xs and the scaled weights are shipped as
  fp8e4m3 hi+lo residual pairs, packed [c][128,2,*] for DoubleRow matmuls.
  rope tables carry sign/partner folds (no 1/sqrt(hd): that is folded into the
  q-side RMS scale on device).  v-bias is folded into a host-computed output
  bias c_out = w_out_loc^T cv.

Device (per core):
  qT/kT = 3-term fp8-DR matmul (XhWh + XlWh + XhWl) + bias   [j, l] bf16
  per-head RMS: ph = onehot-matmul sumsq, rr = recip(sqrt(.)), pb = broadcast
  matmul; rope via partition pair-shuffle; qr = (q*ra + shuf(q)*rb)*pb
  S^T = kr^T qr (bf16) -> exp (Act) -> E^T bf16
  o[lq,65] = sum_lk E^T-chunk^T @ [v|1]  (bf16, M=128-full), TSP-divide by the
  ones column -> o2 [l, j] bf16; DMA-transpose -> oT [j, l]; out = wo^T oT + c_out.
"""

import numpy as np

B, L, D, HD, DC = 2, 2048, 2048, 64, 2048
NH = D // HD
EPS = float(np.finfo(np.float32).eps)
NCORES = 8
JL = 512          # local j per device (8 heads x 64)
NHL = 8           # local heads
SCL2 = float(HD)  # 1/scl^2 with scl = 1/sqrt(HD)


# ---------------------------------------------------------------- host prep
def _f8(a):
    import ml_dtypes
    return np.ascontiguousarray(a.astype(ml_dtypes.float8_e4m3))


def _bf(a):
    import ml_dtypes
    return np.ascontiguousarray(a.astype(ml_dtypes.bfloat16))


def _hf(a):
    return np.ascontiguousarray(a.astype(np.float16))


def _pack_pairs(a):
    """[Dk, N] -> [Dk//256, 128, 2, N] fp8 pair-packed along the contraction."""
    dk, n = a.shape
    return np.ascontiguousarray(a.reshape(dk // 256, 2, 128, n).transpose(0, 2, 1, 3))


def _hilo(a):
    import ml_dtypes
    hi = a.astype(ml_dtypes.float8_e4m3)
    lo = (a - hi.astype(np.float32)).astype(ml_dtypes.float8_e4m3)
    return hi, lo


def _host_prep(x, condition, rope, w_ada, w_qkv, w_out, qk_w):
    import ml_dtypes

    # rope tables: fold qk_w + pair-shuffle sign/partner (no 1/sqrt(hd))
    hd_idx = np.arange(HD)
    sign = np.where(hd_idx % 2 == 0, -1.0, 1.0).astype(np.float32)
    partner = hd_idx ^ 1
    ropeA = (rope[0].T * qk_w[:, None]).astype(np.float32)           # [64, L]
    ropeB = (rope[1].T * (sign * qk_w[partner])[:, None]).astype(np.float32)
    ra = _hf(np.tile(ropeA, (2, 1)))                                  # [128, L]
    rb = _hf(np.tile(ropeB, (2, 1)))

    # head one-hot maps; q-variant folds 1/(HD*scl^2)=1/4096, k-variant 1/HD
    s_t = np.zeros((128, 4, NHL), np.float32)
    for p in range(128):
        for m in range(4):
            s_t[p, m, (m * 128 + p) // HD] = 1.0
    sb = np.zeros((NHL, JL), np.float32)
    for j in range(JL):
        sb[j // HD, j] = 1.0

    # xs = x * rr, per batch; fp8 hi/lo pair-packed [8, 128, 2, L]
    xs8 = []
    for b in range(B):
        rr = 1.0 / np.sqrt(np.mean(x[b] * x[b], axis=-1) + EPS)      # [L]
        xsT = np.ascontiguousarray((x[b] * rr[:, None]).T)           # [D, L]
        hi, lo = _hilo(xsT)
        xs8.append((_pack_pairs(hi), _pack_pairs(lo)))

    in_maps = []
    for dev in range(NCORES):
        b, g = dev // 4, dev % 4
        ss = (w_ada @ condition[b]).astype(np.float32)
        shift, s1 = ss[:D], (1.0 + ss[D:]).astype(np.float32)
        Wq = w_qkv[g * JL:(g + 1) * JL]                              # [512, D]
        Wk = w_qkv[D + g * JL:D + (g + 1) * JL]
        Wv = w_qkv[2 * D + g * JL:2 * D + (g + 1) * JL]
        wo_loc = w_out[:, g * JL:(g + 1) * JL]                       # [D, 512]
        cv = (Wv @ shift).astype(np.float32)                         # [512]
        m = {
            "xs8h": xs8[b][0], "xs8l": xs8[b][1],
            "ra": ra, "rb": rb,
            "s_tq": _hf(s_t), "s_tk": _hf(s_t), "sb": _hf(sb),
            "cq": np.ascontiguousarray((Wq @ shift).astype(np.float32).reshape(4, 128).T),
            "ck": np.ascontiguousarray((Wk @ shift).astype(np.float32).reshape(4, 128).T),
            "cout": np.ascontiguousarray((wo_loc @ cv).astype(np.float32).reshape(16, 128).T),
        }
        woT = np.ascontiguousarray(wo_loc.T)                         # [512, D]
        hi, lo = _hilo(woT)
        m["wo8h"], m["wo8l"] = _pack_pairs(hi), _pack_pairs(lo)      # [2,128,2,2048]
        for nm, W in (("q", Wq), ("k", Wk)):
            WT = np.ascontiguousarray((W * s1[None, :]).T)           # [D, 512]
            hi, lo = _hilo(WT)
            hi_p, lo_p = _pack_pairs(hi), _pack_pairs(lo)            # [8,128,2,512]
            # m-major streaming layout: [32, 128, 2, 128], idx = m*8+c
            m[f"w{nm}8h"] = np.ascontiguousarray(
                hi_p.reshape(8, 128, 2, 4, 128).transpose(3, 0, 1, 2, 4)
            ).reshape(32, 128, 2, 128)
            m[f"w{nm}8l"] = np.ascontiguousarray(
                lo_p.reshape(8, 128, 2, 4, 128).transpose(3, 0, 1, 2, 4)
            ).reshape(32, 128, 2, 128)
        WT = np.ascontiguousarray((Wv * s1[None, :]).T)
        hi, lo = _hilo(WT)
        m["wv8h"], m["wv8l"] = _pack_pairs(hi), _pack_pairs(lo)      # [8,128,2,512]
        in_maps.append(m)
    return in_maps


# ---------------------------------------------------------------- bass build
def _build_nc():
    import concourse.bass as bass
    import concourse.mybir as mybir
    import concourse.tile as tile
    from concourse import bacc

    f32 = mybir.dt.float32
    f16 = mybir.dt.float16
    bf16 = mybir.dt.bfloat16
    f8 = mybir.dt.float8e4
    AF = mybir.ActivationFunctionType
    DR = mybir.MatmulPerfMode.DoubleRow
    DIV = mybir.AluOpType.divide

    nc = bacc.Bacc("TRN2", target_bir_lowering=False, debug=False, num_devices=8)

    xs8_d = {lv: nc.dram_tensor(f"xs8{lv}", [8, 128, 2, L], f8, kind="ExternalInput")
             for lv in ("h", "l")}
    wqk_d = {nm: nc.dram_tensor(nm, [32, 128, 2, 128], f8, kind="ExternalInput")
             for nm in ("wq8h", "wq8l", "wk8h", "wk8l")}
    wv_d = {lv: nc.dram_tensor(f"wv8{lv}", [8, 128, 2, JL], f8, kind="ExternalInput")
            for lv in ("h", "l")}
    ra_d = nc.dram_tensor("ra", [128, L], f16, kind="ExternalInput")
    rb_d = nc.dram_tensor("rb", [128, L], f16, kind="ExternalInput")
    stq_d = nc.dram_tensor("s_tq", [128, 4, NHL], f16, kind="ExternalInput")
    stk_d = nc.dram_tensor("s_tk", [128, 4, NHL], f16, kind="ExternalInput")
    sb_d = nc.dram_tensor("sb", [NHL, JL], f16, kind="ExternalInput")
    cq_d = nc.dram_tensor("cq", [128, 4], f32, kind="ExternalInput")
    ck_d = nc.dram_tensor("ck", [128, 4], f32, kind="ExternalInput")
    cout_d = nc.dram_tensor("cout", [128, 16], f32, kind="ExternalInput")
    wo_d = {lv: nc.dram_tensor(f"wo8{lv}", [2, 128, 2, D], f8, kind="ExternalInput")
            for lv in ("h", "l")}
    out_d = nc.dram_tensor("out", [D, L], bf16, kind="ExternalOutput")

    with tile.TileContext(nc) as tc, \
            nc.allow_low_precision(reason="bf16/fp8 compute"):
        with (
            tc.tile_pool(name="consts", bufs=1) as consts,
            tc.tile_pool(name="qrp", bufs=1) as qrp,
            tc.tile_pool(name="krp", bufs=1) as krp,
            tc.tile_pool(name="vp", bufs=1) as vp,
            tc.tile_pool(name="o2p", bufs=1) as o2p,
            tc.tile_pool(name="etp", bufs=1) as etp,
            tc.tile_pool(name="qkp", bufs=1) as qkp,
            tc.tile_pool(name="srtp", bufs=1) as srtp,
            tc.tile_pool(name="ps", bufs=1, space="PSUM") as ps,
        ):
            # ---------------- consts ----------------
            s_tq = consts.tile([128, 4, NHL], f16)
            nc.sync.dma_start(out=s_tq, in_=stq_d[:, :, :])
            s_tk = s_tq
            sb_t = consts.tile([NHL, JL], f16)
            nc.sync.dma_start(out=sb_t, in_=sb_d[:, :])
            cq_t = consts.tile([128, 4], f32)
            nc.sync.dma_start(out=cq_t, in_=cq_d[:, :])
            ck_t = consts.tile([128, 4], f32)
            nc.sync.dma_start(out=ck_t, in_=ck_d[:, :])
            cout_t = consts.tile([128, 16], f32)
            nc.sync.dma_start(out=cout_t, in_=cout_d[:, :])
            epsq = consts.tile([NHL, 1], f32)
            nc.vector.memset(epsq, EPS * HD)
            epsk = consts.tile([NHL, 1], f32)
            nc.vector.memset(epsk, EPS)


            qr_t = [qrp.tile([128, L], f16, tag=f"qr{m}", name=f"qr{m}")
                    for m in range(4)]
            kr_t = [krp.tile([128, L], f16, tag=f"kr{m}", name=f"kr{m}")
                    for m in range(4)]
            v_t = [vp.tile([128, NHL, HD + 1], bf16, tag=f"v{i}", name=f"v{i}")
                   for i in range(16)]
            o2_t = [o2p.tile([128, JL], bf16, tag=f"o2{i}", name=f"o2{i}")
                    for i in range(16)]

            shuf = [i ^ 1 for i in range(32)]

            with (
                tc.tile_pool(name="xs8p", bufs=1) as xs8p,
                tc.tile_pool(name="w8p", bufs=2) as w8p,
                tc.tile_pool(name="wvp", bufs=1) as wvp,
                tc.tile_pool(name="ropep", bufs=1) as ropep,
                tc.tile_pool(name="rtmp", bufs=1) as rtmp,
            ):
                ra_t = ropep.tile([128, L], f16, tag="ra", name="ra")
                nc.sync.dma_start(out=ra_t, in_=ra_d[:, :])
                rb_t = ropep.tile([128, L], f16, tag="rb", name="rb")
                nc.sync.dma_start(out=rb_t, in_=rb_d[:, :])
                xs8 = {}
                for lv in ("h", "l"):
                    for c in range(8):
                        t = xs8p.tile([128, 2, L], f8, tag=f"xs{lv}{c}",
                                      name=f"xs{lv}{c}")
                        nc.sync.dma_start(out=t, in_=xs8_d[lv][c])
                        xs8[lv, c] = t

                # ---------- q/k projection + per-head RMS + rope ----------
                w_cache = {}

                def emit_qk_lh(nm, m, lh):
                    cbias = cq_t if nm == "q" else ck_t
                    epst = epsq if nm == "q" else epsk
                    lnscale = 1.0 if nm == "q" else 1.0 / HD
                    s_one = s_tq if nm == "q" else s_tk
                    dst = qr_t if nm == "q" else kr_t
                    if (nm, m) not in w_cache:
                        wh, wl = [], []
                        for c in range(8):
                            t = w8p.tile([128, 2, 128], f8, tag=f"w8{2*c}", name="wh")
                            nc.sync.dma_start(out=t, in_=wqk_d[f"w{nm}8h"][m * 8 + c])
                            wh.append(t)
                            t = w8p.tile([128, 2, 128], f8, tag=f"w8{2*c+1}", name="wl")
                            nc.sync.dma_start(out=t, in_=wqk_d[f"w{nm}8l"][m * 8 + c])
                            wl.append(t)
                        w_cache[nm, m] = (wh, wl)
                    wh, wl = w_cache[nm, m]
                    qk = qkp.tile([128, 1024], f16, tag="qk", name=f"{nm}{m}", bufs=1)
                    nsls = [slice(lh * 1024 + n2 * 512, lh * 1024 + (n2 + 1) * 512)
                            for n2 in range(2)]
                    lsls = [slice(n2 * 512, (n2 + 1) * 512) for n2 in range(2)]
                    for n2 in range(2):
                        acc = ps.tile([128, 512], f32, tag="pacc", name="pj", bufs=2)
                        terms = ([("h", wh[c], c) for c in range(8)]
                                 + [("l", wh[c], c) for c in range(8)]
                                 + [("h", wl[c], c) for c in range(8)])
                        for i, (xlv, wt, c) in enumerate(terms):
                            nc.tensor.matmul(
                                acc, lhsT=wt, rhs=xs8[xlv, c][:, :, nsls[n2]],
                                start=(i == 0), stop=(i == len(terms) - 1),
                                perf_mode=DR)
                        nc.vector.tensor_scalar_add(qk[:, lsls[n2]], acc,
                                                    cbias[:, m:m + 1])
                    # per-head sumsq -> rsqrt -> broadcast
                    pbs = []
                    for n2 in range(2):
                        sq = rtmp.tile([128, 512], f16, tag=f"u{n2}",
                                       name="sq", bufs=1)
                        nc.vector.tensor_mul(sq, qk[:, lsls[n2]], qk[:, lsls[n2]])
                        ph = ps.tile([128, 512], f32, tag="pacc", name="ph", bufs=2)
                        nc.tensor.matmul(ph[0:NHL, :], lhsT=s_one[:, m, :],
                                         rhs=sq, start=True, stop=True)
                        srt = srtp.tile([NHL, 512], f16, tag=f"ln{n2}", name="srt")
                        nc.scalar.activation(srt, ph[0:NHL, :], AF.Sqrt,
                                             bias=epst, scale=lnscale)
                        rr = srtp.tile([NHL, 512], f16, tag=f"rr{n2}", name="rr")
                        nc.vector.reciprocal(rr, srt)
                        pb = ps.tile([128, 512], f32, tag="pacc", name="pb", bufs=2)
                        nc.tensor.matmul(pb, lhsT=sb_t[:, m * 128:(m + 1) * 128],
                                         rhs=rr, start=True, stop=True)
                        pbs.append(pb)
                                        # rope: res = (qk*ra + shuf(qk)*rb) * pb; t2 on Pool, hidden
                    t1s, t2s = [], []
                    for n2 in range(2):
                        t1 = rtmp.tile([128, 512], f16, tag=f"t1{n2}",
                                       name="t1", bufs=1)
                        nc.vector.tensor_mul(t1, qk[:, lsls[n2]], ra_t[:, nsls[n2]])
                        qs = rtmp.tile([128, 512], f16, tag=f"qs{n2}",
                                       name="qs", bufs=1)
                        nc.vector.stream_shuffle(qs, qk[:, lsls[n2]], shuf)
                        t2 = rtmp.tile([128, 512], f16, tag=f"t2{n2}",
                                       name="t2", bufs=1)
                        nc.gpsimd.tensor_mul(t2, qs, rb_t[:, nsls[n2]])
                        t1s.append(t1)
                        t2s.append(t2)
                    for n2 in range(2):
                        t3 = rtmp.tile([128, 512], f16, tag=f"u{n2}",
                                       name="t3", bufs=1)
                        nc.vector.tensor_add(t3, t1s[n2], t2s[n2])
                        nc.vector.tensor_mul(dst[m][:, nsls[n2]], t3, pbs[n2])

                # ---------------- v projection ----------------
                wv_tiles = {}

                def emit_v_chunks(lo, hi):
                    if not wv_tiles:
                        for c in range(8):
                            t = wvp.tile([128, 2, JL], f8, tag=f"wv8{2*c}",
                                         name="wvh")
                            nc.sync.dma_start(out=t, in_=wv_d["h"][c])
                            wv_tiles["h", c] = t
                            t = wvp.tile([128, 2, JL], f8, tag=f"wv8{2*c+1}",
                                         name="wvl")
                            nc.sync.dma_start(out=t, in_=wv_d["l"][c])
                            wv_tiles["l", c] = t
                    for mL in range(lo, hi):
                        acc = ps.tile([128, 512], f32, tag="pacc", name="pv",
                                      bufs=2)
                        msl = slice(mL * 128, (mL + 1) * 128)
                        terms = ([("h", wv_tiles["h", c], c) for c in range(8)]
                                 + [("l", wv_tiles["h", c], c) for c in range(8)]
                                 + [("h", wv_tiles["l", c], c) for c in range(8)])
                        for i, (xlv, wt, c) in enumerate(terms):
                            nc.tensor.matmul(
                                acc, lhsT=xs8[xlv, c][:, :, msl], rhs=wt,
                                start=(i == 0), stop=(i == len(terms) - 1),
                                perf_mode=DR)
                        nc.vector.tensor_copy(
                            v_t[mL][:, :, 0:HD],
                            acc.rearrange("p (h d) -> p h d", h=NHL))
                        nc.vector.memset(v_t[mL][:, :, HD:HD + 1], 1.0)

                # ---------------- attention, one head ----------------
                et = {}

                def emit_S_nb(h, nb):
                    m, half = h // 2, h % 2
                    qrh = qr_t[m][half * 64:half * 64 + 64, :]
                    krh = kr_t[m][half * 64:half * 64 + 64, :]
                    for l2c in range(16):
                        psS = ps.tile([128, 1024], f32, tag="psS",
                                      name="psS", bufs=2)
                        for n2 in range(2):
                            nc.tensor.matmul(
                                psS[:, n2 * 512:(n2 + 1) * 512],
                                lhsT=krh[:, l2c * 128:(l2c + 1) * 128],
                                rhs=qrh[:, nb * 1024 + n2 * 512:
                                        nb * 1024 + (n2 + 1) * 512],
                                start=True, stop=True)
                        t = etp.tile([128, 1024], bf16, tag=f"et{l2c}",
                                     name=f"et{l2c}")
                        nc.scalar.activation(t, psS, AF.Exp)
                        et[l2c] = t

                def emit_o_nb(h, nb):
                    for lqt in range(8):
                        po = ps.tile([128, HD + 1], f32, tag="po",
                                     name="po", bufs=2)
                        for l2c in range(16):
                            nc.tensor.matmul(
                                po, lhsT=et[l2c][:, lqt * 128:(lqt + 1) * 128],
                                rhs=v_t[l2c][:, h, :],
                                start=(l2c == 0), stop=(l2c == 15))
                        rd = srtp.tile([128, 1], f32, tag="rd", name="rd", bufs=2)
                        nc.vector.reciprocal(rd, po[:, HD:HD + 1])
                        nc.vector.tensor_scalar_mul(
                            o2_t[nb * 8 + lqt][:, h * 64:(h + 1) * 64],
                            po[:, 0:HD], rd)

                # ---------------- emission order (pipelined) ----------------
                emit_qk_lh("q", 0, 0)
                emit_qk_lh("k", 0, 0)
                emit_qk_lh("k", 0, 1)
                emit_v_chunks(0, 11)
                emit_S_nb(0, 0)
                emit_qk_lh("q", 0, 1)
                emit_v_chunks(11, 16)
                emit_o_nb(0, 0)
                emit_S_nb(0, 1)
                emit_o_nb(0, 1)
                emit_S_nb(1, 0)
                emit_o_nb(1, 0)
                emit_qk_lh("q", 1, 0)
                emit_qk_lh("q", 1, 1)
                emit_S_nb(1, 1)
                emit_o_nb(1, 1)
                emit_qk_lh("k", 1, 0)
                emit_qk_lh("k", 1, 1)
                fillers = [("q", 2), ("k", 2), ("q", 3), ("k", 3)]
                for h in (2, 3, 4, 5):
                    for nb in range(2):
                        emit_S_nb(h, nb)
                        emit_o_nb(h, nb)
                        if nb == 1 and fillers:
                            fnm, fm = fillers.pop(0)
                            emit_qk_lh(fnm, fm, 0)
                            emit_qk_lh(fnm, fm, 1)

            # ------------- transpose + E(6,7) + output projection -------------
            with (
                tc.tile_pool(name="oTp", bufs=1) as oTp,
                tc.tile_pool(name="oT8p", bufs=1) as oT8p,
                tc.tile_pool(name="wop", bufs=1) as wop,
                tc.tile_pool(name="obp", bufs=3) as obp,
            ):
                wo8 = {}
                for lv in ("h", "l"):
                    for jp in range(2):
                        t = wop.tile([128, 2, D], f8, tag=f"wo{lv}{jp}",
                                     name=f"wo{lv}{jp}")
                        nc.sync.dma_start(out=t, in_=wo_d[lv][jp])
                        wo8[lv, jp] = t
                oT_t = [oTp.tile([128, L], bf16, tag=f"oT{jc}", name=f"oT{jc}")
                        for jc in range(4)]
                oT8 = {(lv, jp): oT8p.tile([128, 2, L], f8, tag=f"oT8{lv}{jp}",
                                           name=f"oT8{lv}{jp}")
                       for lv in ("h", "l") for jp in range(2)}

                def emit_transp(jc):
                    for lt in range(16):
                        nc.sync.dma_start(
                            out=oT_t[jc][:, lt * 128:(lt + 1) * 128],
                            in_=o2_t[lt][:, jc * 128:(jc + 1) * 128],
                            transpose=True)

                def emit_hilo(jc, eng):
                    jp, sl = jc // 2, jc % 2
                    eng.tensor_copy(oT8["h", jp][:, sl, :], oT_t[jc])
                    eng.tensor_sub(oT8["l", jp][:, sl, :], oT_t[jc],
                                   oT8["h", jp][:, sl, :])

                emit_transp(0)
                emit_transp(1)
                emit_transp(2)
                emit_hilo(0, nc.gpsimd)
                emit_hilo(1, nc.gpsimd)
                for nb6 in range(2):
                    emit_S_nb(6, nb6)
                    emit_o_nb(6, nb6)
                emit_hilo(2, nc.gpsimd)
                for nb7 in range(2):
                    emit_S_nb(7, nb7)
                    emit_o_nb(7, nb7)
                emit_transp(3)
                emit_hilo(3, nc.vector)

                for m16 in range(16):
                    for lh in range(2):
                        pf = ps.tile([128, 1024], f32, tag="psS", name="pf", bufs=2)
                        for n2 in range(2):
                            terms = [("h", "h"), ("l", "h"), ("h", "l")]
                            mms = [(ov, wv, jp) for jp in range(2)
                                   for (ov, wv) in terms]
                            for i, (ov, wv, jp) in enumerate(mms):
                                nc.tensor.matmul(
                                    pf[:, n2 * 512:(n2 + 1) * 512],
                                    lhsT=wo8[wv, jp][:, :, m16 * 128:(m16 + 1) * 128],
                                    rhs=oT8[ov, jp][:, :, lh * 1024 + n2 * 512:
                                                    lh * 1024 + (n2 + 1) * 512],
                                    start=(i == 0), stop=(i == len(mms) - 1),
                                    perf_mode=DR)
                        ob = obp.tile([128, 1024], bf16, tag="ob", name="ob")
                        nc.vector.tensor_scalar_add(ob, pf, cout_t[:, m16:m16 + 1])
                        nc.sync.dma_start(
                            out=out_d[m16 * 128:(m16 + 1) * 128,
                                      lh * 1024:(lh + 1) * 1024],
                            in_=ob)
    return nc


_NC_CACHE = None


def kernel(**inputs):
    global _NC_CACHE
    from concourse.bass_utils import run_bass_kernel_spmd

    in_maps = _host_prep(
        np.asarray(inputs["x"], np.float32), np.asarray(inputs["condition"], np.float32),
        np.asarray(inputs["rope"], np.float32), np.asarray(inputs["w_ada"], np.float32),
        np.asarray(inputs["w_qkv"], np.float32), np.asarray(inputs["w_out"], np.float32),
        np.asarray(inputs["qk_w"], np.float32))
    if _NC_CACHE is None:
        _NC_CACHE = _build_nc()
        if not _NC_CACHE.is_finalized():
            _NC_CACHE.finalize()
    res = run_bass_kernel_spmd(_NC_CACHE, in_maps, list(range(NCORES)))
    out = np.zeros((B, L, D), np.float32)
    for b in range(B):
        acc = np.zeros((D, L), np.float32)
        for g in range(4):
            acc += res.results[b * 4 + g]["out"].astype(np.float32)
        out[b] = acc.T
    return out
